# revision 26
# baseline (speedup 1.0000x reference)
"""Fused LayerNorm + multi-head attention Trainium2 kernel, 8-core SPMD.

Problem: x[4, 2048, 768] -> LN -> QKV (w_qkv[2304, 768]) -> 12-head attention
         -> out proj (w_out[768, 768] + b_out). f32 I/O, bf16 tensor-engine compute.

Sharding: core c handles batch b=c//2, query-half g=c%2 (1024 queries each).
Each core receives the FULL (rotated) sequence of its batch so K/V are computed
locally -- no collectives. The token order is rotated per-core so the core's own
query chunk is always columns [0, 1024) => identical SPMD program on all cores.

v4 design notes:
- LayerNorm folded into the QKV matmul over RAW bf16 x via two appended
  contraction rows (-mu and 1/rstd); the rstd factor is applied by the DVE
  during PSUM evacuation, fused with the f32->bf16 cast.
- Attention runs in 12 half-query passes (head-pair x query-half of 512).
  Per key tile, the two heads' scores land in one [128,1024] PSUM pair tile
  (head0 cols 0:512 on PE rows 0:64, head1 cols 512:1024 on rows 64:128 --
  row-tiled, so they run concurrently) and are exp'd by a single ACT
  instruction.  U accumulators are [128,512] = 1 PSUM bank each.
- PSUM budget: U pool 2 banks + score pair tiles 4 banks + one 2-bank filler
  slot where K/Q units (during the qh=0 sweep) and the first-half output
  projection (during the qh=1 sweep) run, keeping the PE dense so the HAM
  clock gate stays at 2.4 GHz.
- Softmax reciprocals run on the DVE.  Row->column transposes go through the
  PE (K=1 broadcast matmuls to spread a row across partitions; PE transpose
  mode + a contiguous DRAM bounce to collapse back) -- element-strided DMAs
  measure 10-20us and must be avoided.  No Ln/Exp on the denominators keeps
  ACT on one table set for the whole attention phase.
"""

import numpy as np
import ml_dtypes

import concourse.bass as bass
import concourse.tile as tile
from concourse import bacc, mybir
from concourse.bass_utils import run_bass_kernel_spmd

F32 = mybir.dt.float32
BF16 = mybir.dt.bfloat16
AF = mybir.ActivationFunctionType
ALU = mybir.AluOpType

DIM = 768
HEADS = 12
B, N = 4, 2048
D = 64          # head dim
NQ = 1024       # queries per core
CT = 6          # 768 / 128 channel tiles
NT = 16         # 2048 / 128 token tiles
HP = 6          # head pairs

LAST = None  # BassKernelResults of the most recent run (for test harness)
_NC = None


def build(debug=False):
    nc = bacc.Bacc("TRN2", target_bir_lowering=False, debug=False, num_devices=8)

    xT = nc.dram_tensor("xT", [DIM, N], F32, kind="ExternalInput")
    wqkvT = nc.dram_tensor("wqkvT", [DIM + 2, 3 * DIM], BF16, kind="ExternalInput")
    woutT = nc.dram_tensor("woutT", [DIM + 1, DIM], BF16, kind="ExternalInput")
    ident = nc.dram_tensor("ident", [128, 128], BF16, kind="ExternalInput")
    outT = nc.dram_tensor("outT", [DIM, NQ], F32, kind="ExternalOutput")

    with tile.TileContext(nc) as tc:
        with (
            tc.tile_pool(name="persist", bufs=1) as P1,
            tc.tile_pool(name="work", bufs=2) as PW,
            tc.tile_pool(name="wk2", bufs=2) as PW2,
            tc.tile_pool(name="et", bufs=6) as PE_,
            tc.tile_pool(name="rows", bufs=2) as PR,
            tc.tile_pool(name="psu", bufs=2, space="PSUM") as PU,
            tc.tile_pool(name="psp", bufs=2, space="PSUM") as PP,
            tc.tile_pool(name="psf", bufs=1, space="PSUM") as PF,
            tc.tile_pool(name="dram", bufs=2, space="DRAM") as PD,
        ):
            # ---- persistent SBUF tensors ----
            xb16 = P1.tile([128, CT, N], BF16, tag="xb16")   # raw x, bf16
            xex = P1.tile([2, N], BF16)                      # rows: [-mu ; 1/rstd]
            wq = P1.tile([128, CT, 3 * DIM], BF16)           # W'' rows 0..767
            wex = P1.tile([2, 3 * DIM], BF16)                # W'' rows 768..769 (C; D)
            WO = P1.tile([128, CT, DIM], BF16)               # w_out^T  (f-major tiles)
            wob = P1.tile([1, DIM], BF16)                    # b_out row
            identb = P1.tile([128, 128], BF16)               # identity (PE transpose)
            KT = P1.tile([128, CT, N], BF16)                 # K^T channel-major
            QT = P1.tile([128, CT, NQ], BF16)                # Q^T channel-major
            V4 = P1.tile([128, NT, HEADS, D + 1], BF16)      # V token-major + ones col
            AO = P1.tile([128, CT, NQ], BF16)                # attention out (f-major)
            rstdB = P1.tile([128, N], BF16)                  # rstd bcast to 128 parts
            rstd_col = P1.tile([128, NT], F32)               # rstd token-in-partition
            ones1b = P1.tile([128, 1], BF16)                 # bf16 ones column
            onesr = P1.tile([1, 128], F32)                   # f32 ones row (bcast lhsT)
            onesI = P1.tile([1, 512], BF16)                  # bf16 ones row (bias rhs)
            epsc = P1.tile([1, 1], F32)

            nc.vector.memset(epsc[:], 1e-5)
            nc.vector.memset(ones1b[:], 1.0)
            nc.vector.memset(onesr[:], 1.0)
            nc.vector.memset(onesI[:], 1.0)
            nc.vector.memset(V4[:, :, :, D : D + 1], 1.0)

            # ---- phase A: x load (sync ring) + cast + stats via ones-matmuls ----
            sts = [PP.tile([128, 1024], F32, tag="sp", name=f"st_{h}") for h in range(2)]
            for ct in range(CT):
                for h in range(2):
                    hsl = slice(h * 1024, (h + 1) * 1024)
                    xin = PW.tile([128, 1024], F32, tag="f32w", name=f"xin_{ct}_{h}")
                    # alternate HWDGE rings so x transfers overlap
                    dma_eng = nc.sync if (ct * 2 + h) % 2 == 0 else nc.scalar
                    dma_eng.dma_start(xin[:], xT[ct * 128 : (ct + 1) * 128, hsl])
                    nc.vector.tensor_copy(xb16[:, ct, hsl], xin[:])
                    xsq = PE_.tile([128, 1024], BF16, tag="et", name=f"xsq_{ct}_{h}")
                    nc.vector.tensor_tensor(
                        xsq[:], xb16[:, ct, hsl], xb16[:, ct, hsl], ALU.mult
                    )
                    st = sts[h]
                    for s in range(2):
                        osl = slice(s * 512, (s + 1) * 512)
                        xsl = slice(h * 1024 + s * 512, h * 1024 + s * 512 + 512)
                        nc.tensor.matmul(
                            st[0:1, osl], ones1b[:], xb16[:, ct, xsl],
                            start=(ct == 0), stop=(ct == CT - 1),
                        )
                        nc.tensor.matmul(
                            st[64:65, osl], ones1b[:], xsq[:, osl],
                            start=(ct == 0), stop=(ct == CT - 1),
                        )

            # ---- weight DMAs on the scalar HWDGE ring (parallel with x) ----
            for ct in range(CT):
                nc.scalar.dma_start(wq[:, ct, :], wqkvT[ct * 128 : (ct + 1) * 128, :])
                nc.scalar.dma_start(WO[:, ct, :], woutT[ct * 128 : (ct + 1) * 128, :])
            nc.scalar.dma_start(wex[:], wqkvT[DIM : DIM + 2, :])
            nc.scalar.dma_start(wob[:], woutT[DIM : DIM + 1, :])
            nc.scalar.dma_start(identb[:], ident[:, :])

            # ---- QKV unit emitters (raw x matmul + fused postscale) ----
            pool_tag = {id(PP): "sp", id(PF): "fill", id(PU): "u"}

            def kq_unit_gen(which, hp, nh, pool):
                base = DIM if which == "K" else 0
                fsl = slice(base + hp * 128, base + (hp + 1) * 128)
                acc = pool.tile(
                    [128, 1024], F32, tag=pool_tag[id(pool)],
                    name=f"kq{which}_{hp}_{nh}",
                )
                # ct-outer so the xex-dependent matmuls are the last yields
                for ct in range(CT + 1):
                    for s in range(2):
                        psl = slice(s * 512, (s + 1) * 512)
                        nsl = slice(nh * 1024 + s * 512, nh * 1024 + s * 512 + 512)
                        nc.tensor.matmul(
                            acc[:, psl],
                            wq[:, ct, fsl] if ct < CT else wex[:, fsl],
                            xb16[:, ct, nsl] if ct < CT else xex[:, nsl],
                            start=(ct == 0), stop=(ct == CT),
                        )
                        yield
                dst = KT if which == "K" else QT
                nsl = slice(nh * 1024, (nh + 1) * 1024)
                nc.vector.tensor_tensor(
                    dst[:, hp, nsl], acc[:], rstdB[:, nsl], ALU.mult
                )

            class Dribbler:
                """Interleave filler-unit instructions between attention
                matmuls so the PE queue never runs a long filler burst that
                stalls the exp stream."""

                def __init__(self):
                    self.gens = []

                def add(self, gen):
                    self.gens.append(gen)

                def emit(self, n):
                    while n > 0 and self.gens:
                        try:
                            next(self.gens[0])
                            n -= 1
                        except StopIteration:
                            self.gens.pop(0)

                def emit_rr(self, n):
                    # round-robin across units: keeps dependency-stalling
                    # instructions (e.g. xex-gated matmuls) last in the queue
                    while n > 0 and self.gens:
                        g = self.gens.pop(0)
                        try:
                            next(g)
                            n -= 1
                            self.gens.append(g)
                        except StopIteration:
                            pass

                def drain(self):
                    self.emit(10**9)

            def emit_kq(hp, pools=(PF, PF, PF)):
                for gen in (
                    kq_unit_gen("Q", hp, 0, pools[0]),
                    kq_unit_gen("K", hp, 0, pools[1]),
                    kq_unit_gen("K", hp, 1, pools[2]),
                ):
                    for _ in gen:
                        pass

            def v_unit(nt, pool):
                nsl = slice(nt * 128, (nt + 1) * 128)
                acc = pool.tile(
                    [128, 1024], F32, tag=pool_tag[id(pool)], name=f"v_{nt}"
                )
                for ct in range(CT + 1):
                    # matmul output must not straddle a 2KB PSUM bank: 768 = 512+256
                    for lo, sz in ((0, 512), (512, 256)):
                        fsl = slice(2 * DIM + lo, 2 * DIM + lo + sz)
                        nc.tensor.matmul(
                            acc[:, lo : lo + sz],
                            xb16[:, ct, nsl] if ct < CT else xex[:, nsl],
                            wq[:, ct, fsl] if ct < CT else wex[:, fsl],
                            start=(ct == 0), stop=(ct == CT),
                        )
                nc.vector.tensor_scalar(
                    V4[:, nt, :, 0:D],
                    acc[:, 0:DIM].rearrange("p (h d) -> p h d", h=HEADS),
                    rstd_col[:, nt : nt + 1],
                    None,
                    ALU.mult,
                )

            def proj_unit_gen(ot, qh, pool):
                osl = slice(ot * 128, (ot + 1) * 128)
                qsl = slice(qh * 512, (qh + 1) * 512)
                acc = pool.tile(
                    [128, 512], F32, tag=pool_tag[id(pool)], name=f"po_{ot}_{qh}"
                )
                nc.tensor.matmul(
                    acc[:, 0:512], wob[:, osl], onesI[:], start=True, stop=False
                )
                yield
                for ft in range(CT):
                    nc.tensor.matmul(
                        acc[:, 0:512], WO[:, ft, osl], AO[:, ft, qsl],
                        start=False, stop=(ft == CT - 1),
                    )
                    yield
                outsb = PW.tile([128, 512], F32, tag="f32w", name=f"outsb_{ot}_{qh}")
                nc.vector.tensor_copy(outsb[:], acc[:, 0:512])
                nc.sync.dma_start(outT[osl, qsl], outsb[:])

            def proj_unit(ot, qh, pool):
                for _ in proj_unit_gen(ot, qh, pool):
                    pass


            # ---- rows: mu, var, rstd (f32), 1/rstd + -mu (bf16 x-extra rows) ----
            # var = E[x^2] - mu^2 with a fused STT so the stats PSUM tiles are
            # read exactly twice and release early (frees slots for K/Q(0) raw)
            r_mu = PR.tile([1, N], F32, tag="row", name="r_mu")
            r_var = PR.tile([1, N], F32, tag="row", name="r_var")
            for h in range(2):
                hsl = slice(h * 1024, (h + 1) * 1024)
                nc.vector.tensor_scalar_mul(r_mu[:, hsl], sts[h][0:1, :], 1.0 / DIM)
                nc.vector.tensor_tensor(
                    r_var[:, hsl], r_mu[:, hsl], r_mu[:, hsl], ALU.mult
                )
                nc.vector.scalar_tensor_tensor(
                    r_var[:, hsl], sts[h][64:65, :], 1.0 / DIM, r_var[:, hsl],
                    ALU.mult, ALU.subtract,
                )
                # per-half Ln so it overlaps the other half's DVE ops
                nc.scalar.activation(
                    r_var[:, hsl], r_var[:, hsl], AF.Ln, bias=epsc[:]
                )
            # K/Q(0): dribble the raw-x matmuls (first 12 yields per unit)
            # under the rest of the rows chain; stats slots are free by now.
            # Pools all PP so PF stays free for pcol below (psb is on PU).
            kq0 = Dribbler()
            kq0.add(kq_unit_gen("Q", 0, 0, PP))
            kq0.add(kq_unit_gen("K", 0, 0, PP))
            kq0.add(kq_unit_gen("K", 0, 1, PP))
            kq0.emit_rr(36)

            # engines write at 32-aligned partition bases only: stage rstdinv at
            # partition 0 and DMA-shift into xex row 1
            for h in range(2):
                hsl = slice(h * 1024, (h + 1) * 1024)
                rinv = PW2.tile([1, 1024], BF16, tag="rrb", name=f"rinv_{h}")
                nc.scalar.activation(rinv[:], r_var[:, hsl], AF.Exp, scale=0.5)
                nc.sync.dma_start(xex[1:2, hsl], rinv[:])
                nc.scalar.activation(r_var[:, hsl], r_var[:, hsl], AF.Exp, scale=-0.5)
                nc.vector.tensor_scalar_mul(xex[0:1, hsl], r_mu[:, hsl], -1.0)

            # rstd broadcast to all 128 partitions (for K/Q postscale); PU pool
            # so it cannot cycle with the K/Q(0) accumulators on PP
            for h in range(2):
                for s in range(2):
                    sl = slice(h * 1024 + s * 512, h * 1024 + s * 512 + 512)
                    psb = PU.tile([128, 512], F32, tag="u", name=f"rb_{h}_{s}")
                    nc.tensor.matmul(psb[:], onesr[:], r_var[:, sl])
                    nc.vector.tensor_copy(rstdB[:, sl], psb[:])
            # rstd token-in-partition (for V postscale): K=1 broadcast matmuls
            pcol = PF.tile([128, NT], F32, tag="fill", name="rstdcol_ps")
            for t in range(NT):
                nc.tensor.matmul(
                    pcol[:, t : t + 1],
                    r_var[0:1, t * 128 : (t + 1) * 128],
                    onesr[0:1, 0:1],
                )
            nc.vector.tensor_copy(rstd_col[:], pcol[:])
            kq0.drain()

            for nt in range(NT):
                v_unit(nt, (PF, PP, PP)[nt % 3])

            # ---- attention: 12 half-query passes, hp-major so each next
            # head pair's K/Q units spread over TWO passes (2 matmuls per
            # key-tile step keeps every pass near the ACT exp cadence) ----
            scale = float(D) ** -0.5
            drib = Dribbler()

            def den_chain_gen(hp, qh, qsl, den, Uev0, Uev1):
                # softmax reciprocal, all off ACT: K=1 broadcast matmuls
                # (row -> partitions) -> DVE reciprocal -> PE transpose ->
                # contiguous DRAM bounce -> gpsimd broadcast.  PE ops yield so
                # the chain dribbles into the next pass instead of bursting.
                dps = PF.tile([128, 8], F32, tag="fill", name=f"dps_{hp}_{qh}")
                for t in range(8):
                    nc.tensor.matmul(
                        dps[:, t : t + 1],
                        den[0:1, t * 128 : (t + 1) * 128],
                        onesr[0:1, 0:1],
                    )
                    yield
                rcolf = PW.tile([128, 8], F32, tag="rcolf", name=f"rcolf_{hp}_{qh}")
                nc.vector.reciprocal(rcolf[:], dps[:])
                rcb = PW.tile([128, 8], BF16, tag="rcb", name=f"rcb_{hp}_{qh}")
                nc.vector.tensor_copy(rcb[:], rcolf[:])
                tps = PF.tile([8, 128], BF16, tag="fill", name=f"tps_{hp}_{qh}")
                nc.tensor.transpose(tps[:], rcb[:], identb[:])
                yield
                rct = PW.tile([8, 128], BF16, tag="rct", name=f"rct_{hp}_{qh}")
                nc.vector.tensor_copy(rct[:], tps[:])
                ddr = PD.tile([1, 1024], BF16, name=f"ddr_{hp}_{qh}")
                nc.sync.dma_start(
                    ddr[0:1, :].rearrange("o (t p) -> (o t) p", t=8), rct[:]
                )
                rrb = PW2.tile([1, 1024], BF16, tag="rrb", name=f"rrb_{hp}_{qh}")
                nc.sync.dma_start(rrb[0:1, 0:1024], ddr[:])
                rbB = PW2.tile([64, 1024], BF16, tag="rbB", name=f"rbB_{hp}_{qh}")
                nc.gpsimd.partition_broadcast(rbB[:], rrb[0:1, 0:1024])
                nc.vector.tensor_tensor(
                    AO[0:64, hp, qsl], Uev0[:], rbB[:, 0:512], ALU.mult
                )
                AOtmp = PW2.tile([64, 512], BF16, tag="AOtmp", name=f"AOtmp_{hp}_{qh}")
                nc.vector.tensor_tensor(
                    AOtmp[:], Uev1[:], rbB[:, 512:1024], ALU.mult
                )
                nc.sync.dma_start(AO[64:128, hp, qsl], AOtmp[:])

            for hp in range(HP):
                for qh in range(2):
                    qsl = slice(qh * 512, (qh + 1) * 512)
                    if qh == 0 and hp + 1 < HP:
                        drib.add(kq_unit_gen("Q", hp + 1, 0, PF))
                        drib.add(kq_unit_gen("K", hp + 1, 0, PF))
                        drib.add(kq_unit_gen("K", hp + 1, 1, PF))
                    if hp == HP - 1 and qh == 1:
                        for ot in range(CT):
                            drib.add(proj_unit_gen(ot, 0, PF))
                    U0 = PU.tile([128, 512], F32, tag="u", name=f"U0_{hp}_{qh}")
                    U1 = PU.tile([128, 512], F32, tag="u", name=f"U1_{hp}_{qh}")
                    for jt in range(NT):
                        jsl = slice(jt * 128, (jt + 1) * 128)
                        sp = PP.tile([128, 1024], F32, tag="sp", name=f"sp_{hp}_{qh}_{jt}")
                        # the pair's heads on disjoint PE row groups -> concurrent
                        nc.tensor.matmul(
                            sp[:, 0:512], KT[0:64, hp, jsl], QT[0:64, hp, qsl],
                            start=True, stop=True,
                        )
                        nc.tensor.matmul(
                            sp[:, 512:1024], KT[64:128, hp, jsl], QT[64:128, hp, qsl],
                            start=True, stop=True,
                        )
                        ET = PE_.tile([128, 1024], BF16, tag="et", name=f"ET_{hp}_{qh}_{jt}")
                        nc.scalar.activation(ET[:], sp[:], AF.Exp, scale=scale)
                        nc.tensor.matmul(
                            U0[0 : D + 1, :], V4[:, jt, 2 * hp, :], ET[:, 0:512],
                            start=(jt == 0), stop=(jt == NT - 1),
                        )
                        nc.tensor.matmul(
                            U1[0 : D + 1, :], V4[:, jt, 2 * hp + 1, :], ET[:, 512:1024],
                            start=(jt == 0), stop=(jt == NT - 1),
                        )
                        drib.emit(4 if (hp == HP - 1 and qh == 1) else 2)
                    # evacuate U to SBUF right away -- releases the PSUM banks
                    # ~1us after the last AV so the next pass's AVs are not
                    # gated on the denominator chain
                    Uev0 = PW2.tile([64, 512], BF16, tag="uev0", name=f"Uev0_{hp}_{qh}")
                    Uev1 = PW2.tile([64, 512], BF16, tag="uev1", name=f"Uev1_{hp}_{qh}")
                    den = PR.tile([1, 1024], F32, tag="drow", name=f"den_{hp}_{qh}")
                    nc.vector.tensor_copy(Uev0[:], U0[0:D, :])
                    nc.vector.tensor_copy(Uev1[:], U1[0:D, :])
                    nc.vector.tensor_copy(den[0:1, 0:512], U0[D : D + 1, :])
                    nc.vector.tensor_copy(den[0:1, 512:1024], U1[D : D + 1, :])
                    drib.add(den_chain_gen(hp, qh, qsl, den, Uev0, Uev1))
            drib.drain()
            for ot in range(CT):
                proj_unit(ot, 1, (PF, PP, PU)[ot % 3])

    nc.finalize()
    return nc


def _get_nc():
    global _NC
    if _NC is None:
        _NC = build()
    return _NC


def kernel(x, ln_w, ln_b, w_qkv, w_out, b_out):
    global LAST
    x = np.asarray(x, dtype=np.float32)
    ln_w = np.asarray(ln_w, dtype=np.float32)
    ln_b = np.asarray(ln_b, dtype=np.float32)
    w_qkv = np.asarray(w_qkv, dtype=np.float32)
    w_out = np.asarray(w_out, dtype=np.float32)
    b_out = np.asarray(b_out, dtype=np.float32)

    bf16 = ml_dtypes.bfloat16
    # W'' = [ (w_qkv * ln_w)^T ; rowsum of (w_qkv*ln_w) ; w_qkv @ ln_b ]
    wprime = w_qkv * ln_w[None, :]
    wqkvT = np.concatenate(
        [wprime.T, wprime.sum(axis=1)[None, :], (w_qkv @ ln_b)[None, :]], axis=0
    ).astype(bf16)
    woutT = np.concatenate([w_out.T, b_out[None, :]], axis=0).astype(bf16)
    identm = np.eye(128, dtype=bf16)

    in_maps = []
    for c in range(8):
        b, g = c // 2, c % 2
        order = np.r_[g * NQ : (g + 1) * NQ, (1 - g) * NQ : (2 - g) * NQ]
        xTc = np.ascontiguousarray(x[b][order].T)
        in_maps.append({"xT": xTc, "wqkvT": wqkvT, "woutT": woutT, "ident": identm})

    nc = _get_nc()
    LAST = run_bass_kernel_spmd(nc, in_maps, core_ids=list(range(8)))

    out = np.empty((B, N, DIM), dtype=np.float32)
    for c in range(8):
        b, g = c // 2, c % 2
        out[b, g * NQ : (g + 1) * NQ, :] = LAST.results[c]["outT"].T
    return out
